# revision 2
# baseline (speedup 1.0000x reference)
"""Multi-head attention (B=2, N=2048, D=768, H=12) on 8 Trainium2 NeuronCores.

Sharding: data-parallel over rows of (B*N) with redundant K/V projection.
Each core c handles batch b=c//4 and query rows q0=(c%4)*512 .. q0+512.
It receives the full batch's x (rolled so its own query rows come first --
softmax over keys is permutation invariant, so key order doesn't matter),
computes K/V for the whole batch (4x redundant) plus Q for its own rows,
runs attention + output projection for its rows, and returns [512, 768].
No cross-core communication is needed.

Device-side layout strategy:
  - x is transposed on the PE (via identity matmuls) into x^T [768, seq]
    d-major tiles, which serve as rhs (K^T/Q^T projections) and lhsT (V).
  - K^T, Q^T are produced directly in [d, seq] layout (fp32r) so the
    scores matmul needs no further transposes; scores come out as
    scores^T [keys, q] in PSUM, exp runs on ACT (scale folded in,
    no max-subtraction needed at these magnitudes), P stored bf16.
  - V tiles are stored bf16 as [V_h0 | 1 | V_h1 | 1] per head-pair so a
    single M=65 matmul per key-chunk yields attnout^T rows 0..63 and the
    softmax denominator (row 64) for free.
  - attnout^T is normalized per head (reciprocal + gpsimd partition
    broadcast) straight into the lhsT layout the output projection needs.
All matmuls run as fp32r (full PE rate at N>=256) except AV (bf16).
"""

import sys

sys.path.insert(0, "/opt/trn_rl_repo")

import numpy as np

import concourse.bass as bass
import concourse.mybir as mybir
import concourse.tile as tile
from concourse import bacc
from concourse import bass_utils
from concourse.masks import make_identity

B, N, D = 2, 2048, 768
H, DH = 12, 64
NCORES = 8
S = 2048          # keys per batch
SQ = 512          # query rows per core
NSC = S // 512    # seq chunks (of 512) for the projections
NKC = S // 128    # key chunks (of 128) for attention
NPAIR = H // 2    # head pairs
KC = D // 128     # contraction chunks
SCALE = DH ** -0.5

f32 = mybir.dt.float32
f32r = mybir.dt.float32r
bf16 = mybir.dt.bfloat16
ADD = mybir.AluOpType.add
EXP = mybir.ActivationFunctionType.Exp

_CACHE = {}


def _build():
    nc = bacc.Bacc("TRN2", target_bir_lowering=False, debug=False,
                   enable_asserts=False, num_devices=NCORES)
    xb = nc.dram_tensor("xb", [S, D], f32, kind="ExternalInput").ap()
    wqkv = nc.dram_tensor("wqkv", [D, 3 * D], f32, kind="ExternalInput").ap()
    bqkv = nc.dram_tensor("bqkv", [3 * D], f32, kind="ExternalInput").ap()
    wproj = nc.dram_tensor("wproj", [D, D], f32, kind="ExternalInput").ap()
    bproj = nc.dram_tensor("bproj", [D], f32, kind="ExternalInput").ap()
    out = nc.dram_tensor("out", [SQ, D], f32, kind="ExternalOutput").ap()

    with tile.TileContext(nc) as tc:
        with tc.tile_pool(name="consts", bufs=1) as consts, \
             tc.tile_pool(name="wv_pool", bufs=1) as wv_pool, \
             tc.tile_pool(name="kt_pool", bufs=1) as kt_pool, \
             tc.tile_pool(name="qt_pool", bufs=1) as qt_pool, \
             tc.tile_pool(name="v_pool", bufs=1) as v_pool, \
             tc.tile_pool(name="at_pool", bufs=1) as at_pool, \
             tc.tile_pool(name="ws_pool", bufs=2) as ws_pool, \
             tc.tile_pool(name="xn_pool", bufs=6) as xn_pool, \
             tc.tile_pool(name="xt_pool", bufs=2) as xt_pool, \
             tc.tile_pool(name="p_pool", bufs=2) as p_pool, \
             tc.tile_pool(name="nrm_pool", bufs=2) as nrm_pool, \
             tc.tile_pool(name="wp_pool", bufs=1) as wp_pool, \
             tc.tile_pool(name="outp", bufs=2) as outp, \
             tc.tile_pool(name="ps1", bufs=4, space="PSUM") as ps1, \
             tc.tile_pool(name="ps2", bufs=2, space="PSUM") as ps2:

            # ---- constants ----
            ident = consts.tile([128, 128], f32)
            make_identity(nc, ident)
            # bqkv as [128, 18]: col j holds bqkv[128j .. 128j+127]
            bq_sb = consts.tile([128, 18], f32)
            nc.sync.dma_start(out=bq_sb, in_=bqkv.rearrange("(j p) -> p j", p=128))
            # bproj broadcast to all partitions
            bp_bc = consts.tile([128, D], f32)
            bp_in = bass.AP(tensor=bproj.tensor, offset=bproj.offset,
                            ap=[[0, 128]] + list(bproj.ap))
            nc.gpsimd.dma_start(out=bp_bc, in_=bp_in)

            # ---- persistent operand tiles ----
            wv = []
            for c in range(KC):
                wvt = wv_pool.tile([128, D], f32r, name=f"wv{c}", tag=f"wv{c}")
                nc.sync.dma_start(
                    out=wvt, in_=wqkv[c * 128:(c + 1) * 128, 2 * D:3 * D].bitcast(f32r))
                wv.append(wvt)
            wp = []
            for c in range(KC):
                wpt = wp_pool.tile([128, D], f32r, name=f"wp{c}", tag=f"wp{c}")
                nc.sync.dma_start(
                    out=wpt, in_=wproj[c * 128:(c + 1) * 128, :].bitcast(f32r))
                wp.append(wpt)
            kt = [kt_pool.tile([128, S], f32r, name=f"kt{j}", tag=f"kt{j}")
                  for j in range(NPAIR)]
            qt = [qt_pool.tile([128, SQ], f32r, name=f"qt{j}", tag=f"qt{j}")
                  for j in range(NPAIR)]
            vt = [v_pool.tile([128, NPAIR * 130], bf16, name=f"vt{k}", tag=f"vt{k}")
                  for k in range(NKC)]
            at = [at_pool.tile([128, SQ], f32r, name=f"at{j}", tag=f"at{j}")
                  for j in range(NPAIR)]

            # ones columns of the V tiles (col 64 and 129 of each pair block)
            for k in range(NKC):
                nc.vector.memset(
                    vt[k].rearrange("p (j t h) -> p j t h", j=NPAIR, t=2)[:, :, :, 64:65],
                    1.0)

            # ---- phase A: x^T, then Q/K/V projections, seq chunk by seq chunk ----
            for s in range(NSC):
                # load x rows and transpose into d-major tiles
                xns = []
                for j in range(4):
                    xn = xn_pool.tile([128, D], f32, name=f"xn{s}_{j}", tag="xn")
                    nc.sync.dma_start(
                        out=xn, in_=xb[s * 512 + j * 128: s * 512 + (j + 1) * 128, :])
                    xns.append(xn)
                xts = []
                for c in range(KC):
                    tp = ps1.tile([128, 512], f32, name=f"tp{s}_{c}", tag="ps1")
                    for j in range(4):
                        nc.tensor.transpose(
                            tp[:, j * 128:(j + 1) * 128],
                            xns[j][:, c * 128:(c + 1) * 128], ident[:])
                    xtc = xt_pool.tile([128, 512], f32r, name=f"xt{s}_{c}", tag=f"xt{c}")
                    nc.vector.tensor_copy(xtc, tp)
                    xts.append(xtc)

                # Q^T (own rows are the first seq chunk thanks to the host-side roll)
                if s == 0:
                    for j in range(NPAIR):
                        wq = ws_pool.tile([128, KC, 128], f32r, name=f"wq{j}", tag="ws")
                        nc.sync.dma_start(
                            out=wq,
                            in_=wqkv[:, j * 128:(j + 1) * 128]
                            .rearrange("(c p) m -> p c m", p=128).bitcast(f32r))
                        qp = ps1.tile([128, 512], f32, name=f"qp{j}", tag="ps1")
                        for c in range(KC):
                            nc.tensor.matmul(qp, wq[:, c, :], xts[c][:],
                                             start=(c == 0), stop=(c == KC - 1))
                        nc.vector.tensor_scalar_add(qt[j], qp, bq_sb[:, j:j + 1])

                # K^T columns for this seq chunk
                for j in range(NPAIR):
                    wk = ws_pool.tile([128, KC, 128], f32r, name=f"wk{j}_{s}", tag="ws")
                    nc.sync.dma_start(
                        out=wk,
                        in_=wqkv[:, D + j * 128:D + (j + 1) * 128]
                        .rearrange("(c p) m -> p c m", p=128).bitcast(f32r))
                    kp = ps1.tile([128, 512], f32, name=f"kp{j}_{s}", tag="ps1")
                    for c in range(KC):
                        nc.tensor.matmul(kp, wk[:, c, :], xts[c][:],
                                         start=(c == 0), stop=(c == KC - 1))
                    nc.vector.tensor_scalar_add(
                        kt[j][:, s * 512:(s + 1) * 512], kp, bq_sb[:, 6 + j:7 + j])

                # V rows for this seq chunk (all heads), into paired bf16 layout
                for m in range(4):
                    k = s * 4 + m
                    vp = ps2.tile([128, 1024], f32, name=f"vp{k}", tag="ps2")
                    for c in range(KC):
                        nc.tensor.matmul(vp[:, 0:512],
                                         xts[c][:, m * 128:(m + 1) * 128],
                                         wv[c][:, 0:512],
                                         start=(c == 0), stop=(c == KC - 1))
                    for c in range(KC):
                        nc.tensor.matmul(vp[:, 512:768],
                                         xts[c][:, m * 128:(m + 1) * 128],
                                         wv[c][:, 512:768],
                                         start=(c == 0), stop=(c == KC - 1))
                    nc.vector.tensor_copy(
                        vt[k].rearrange("p (j t h) -> p j t h",
                                        j=NPAIR, t=2)[:, :, :, 0:64],
                        vp[:, 0:768].rearrange("p (j t h) -> p j t h", j=NPAIR, t=2))

            # ---- phase B: attention, head by head ----
            for h in range(H):
                j, half = h // 2, h % 2
                po = half * 64
                av = ps1.tile([128, 512], f32, name=f"av{h}", tag="ps1")
                for sl in range(NKC // 2):
                    sc = ps2.tile([128, 1024], f32, name=f"sc{h}_{sl}", tag="ps2")
                    for i in range(2):
                        k = sl * 2 + i
                        nc.tensor.matmul(
                            sc[:, i * 512:(i + 1) * 512],
                            kt[j][po:po + 64, k * 128:(k + 1) * 128],
                            qt[j][po:po + 64, :], start=True, stop=True)
                    pt = p_pool.tile([128, 1024], bf16, name=f"p{h}_{sl}", tag="p")
                    nc.scalar.activation(pt, sc, EXP, scale=SCALE)
                    for i in range(2):
                        k = sl * 2 + i
                        nc.tensor.matmul(
                            av[0:65, :],
                            vt[k][:, j * 130 + half * 65: j * 130 + half * 65 + 65],
                            pt[:, i * 512:(i + 1) * 512],
                            start=(k == 0), stop=(k == NKC - 1))
                # normalize into attnout^T (and add v-bias; exact since
                # (V+1 b_v)^T P / sums = V^T P / sums + b_v)
                sums = nrm_pool.tile([1, 512], f32, name=f"sums{h}", tag="sums")
                nc.vector.tensor_copy(sums, av[64:65, :])
                rs = nrm_pool.tile([1, 512], f32, name=f"rs{h}", tag="rs")
                nc.vector.reciprocal(rs, sums)
                bc = nrm_pool.tile([64, 512], f32, name=f"bc{h}", tag="bc")
                nc.gpsimd.partition_broadcast(bc, rs[:])
                dst = at[j][po:po + 64, :]
                nc.vector.tensor_mul(dst, av[0:64, :], bc[:])
                nc.vector.tensor_scalar_add(dst, dst, bq_sb[po:po + 64, 12 + j:13 + j])

            # ---- phase C: output projection ----
            for m in range(4):
                pp = ps2.tile([128, 1024], f32, name=f"pp{m}", tag="ps2")
                for c in range(KC):
                    nc.tensor.matmul(pp[:, 0:512],
                                     at[c][:, m * 128:(m + 1) * 128],
                                     wp[c][:, 0:512],
                                     start=(c == 0), stop=(c == KC - 1))
                for c in range(KC):
                    nc.tensor.matmul(pp[:, 512:768],
                                     at[c][:, m * 128:(m + 1) * 128],
                                     wp[c][:, 512:768],
                                     start=(c == 0), stop=(c == KC - 1))
                ot = outp.tile([128, D], f32, name=f"ot{m}", tag="ot")
                nc.vector.tensor_tensor(ot, pp[:, 0:768], bp_bc[:], ADD)
                nc.sync.dma_start(out=out[m * 128:(m + 1) * 128, :], in_=ot)

    nc.compile()
    return nc


def get_nc():
    if "nc" not in _CACHE:
        _CACHE["nc"] = _build()
    return _CACHE["nc"]


def make_in_maps(x, W_qkv, b_qkv, W_proj, b_proj):
    x = np.ascontiguousarray(np.asarray(x, dtype=np.float32))
    W_qkv = np.ascontiguousarray(np.asarray(W_qkv, dtype=np.float32))
    b_qkv = np.ascontiguousarray(np.asarray(b_qkv, dtype=np.float32))
    W_proj = np.ascontiguousarray(np.asarray(W_proj, dtype=np.float32))
    b_proj = np.ascontiguousarray(np.asarray(b_proj, dtype=np.float32))
    in_maps = []
    for c in range(NCORES):
        b, q0 = c // 4, (c % 4) * SQ
        xbat = np.roll(x[b], -q0, axis=0)  # own query rows first; key order is free
        in_maps.append({"xb": np.ascontiguousarray(xbat), "wqkv": W_qkv,
                        "bqkv": b_qkv, "wproj": W_proj, "bproj": b_proj})
    return in_maps


def run(in_maps, **kw):
    return bass_utils.run_bass_kernel_spmd(get_nc(), in_maps,
                                           core_ids=list(range(NCORES)), **kw)


def kernel(x, W_qkv, b_qkv, W_proj, b_proj):
    in_maps = make_in_maps(x, W_qkv, b_qkv, W_proj, b_proj)
    res = run(in_maps)
    out = np.empty((B, N, D), dtype=np.float32)
    for c in range(NCORES):
        b, q0 = c // 4, (c % 4) * SQ
        out[b, q0:q0 + SQ] = res.results[c]["out"]
    return out


# revision 6
# speedup vs baseline: 1.0274x; 1.0274x over previous
"""Multi-head attention (B=2, N=2048, D=768, H=12) on 8 Trainium2 NeuronCores.

Sharding: data-parallel over rows of (B*N) with redundant K/V projection.
Each core c handles batch b=c//4 and query rows q0=(c%4)*512 .. q0+512.
It receives the full batch's x (rolled so its own query rows come first --
softmax over keys is permutation invariant, so key order doesn't matter),
computes K/V for the whole batch (4x redundant) plus Q for its own rows,
runs attention + output projection for its rows, and returns [512, 768].
No cross-core communication is needed.

Device-side layout strategy:
  - x is transposed on the PE (via identity matmuls) into x^T [768, seq]
    d-major tiles, which serve as rhs (K^T/Q^T projections) and lhsT (V).
  - K^T, Q^T are produced directly in [d, seq] layout (fp32r) so the
    scores matmul needs no further transposes; scores come out as
    scores^T [keys, q] in PSUM, exp runs on ACT (scale folded in,
    no max-subtraction needed at these magnitudes), P stored bf16.
  - V tiles are stored bf16 as [V_h0 | 1 | V_h1 | 1] per head-pair so a
    single M=65 matmul per key-chunk yields attnout^T rows 0..63 and the
    softmax denominator (row 64) for free.
  - attnout^T is normalized per head (reciprocal + gpsimd partition
    broadcast) straight into the lhsT layout the output projection needs.
All matmuls run as fp32r (full PE rate at N>=256) except AV (bf16).
"""

import sys

sys.path.insert(0, "/opt/trn_rl_repo")

import numpy as np

import concourse.bass as bass
import concourse.mybir as mybir
import concourse.tile as tile
from concourse import bacc
from concourse import bass_utils
from concourse.masks import make_identity

B, N, D = 2, 2048, 768
H, DH = 12, 64
NCORES = 8
S = 2048          # keys per batch
SQ = 512          # query rows per core
NSC = S // 512    # seq chunks (of 512) for the projections
NKC = S // 128    # key chunks (of 128) for attention
NPAIR = H // 2    # head pairs
KC = D // 128     # contraction chunks
SCALE = DH ** -0.5

f32 = mybir.dt.float32
f32r = mybir.dt.float32r
bf16 = mybir.dt.bfloat16
ADD = mybir.AluOpType.add
EXP = mybir.ActivationFunctionType.Exp

_CACHE = {}


def _build():
    nc = bacc.Bacc("TRN2", target_bir_lowering=False, debug=False,
                   enable_asserts=False, num_devices=NCORES)
    xb = nc.dram_tensor("xb", [S, D], f32, kind="ExternalInput").ap()
    wqkv = nc.dram_tensor("wqkv", [D, 3 * D], f32, kind="ExternalInput").ap()
    bqkv = nc.dram_tensor("bqkv", [3 * D], f32, kind="ExternalInput").ap()
    wproj = nc.dram_tensor("wproj", [D, D], f32, kind="ExternalInput").ap()
    bproj = nc.dram_tensor("bproj", [D], f32, kind="ExternalInput").ap()
    out = nc.dram_tensor("out", [SQ, D], f32, kind="ExternalOutput").ap()

    with tile.TileContext(nc) as tc:
        with tc.tile_pool(name="consts", bufs=1) as consts, \
             tc.tile_pool(name="wv_pool", bufs=1) as wv_pool, \
             tc.tile_pool(name="kt_pool", bufs=1) as kt_pool, \
             tc.tile_pool(name="qt_pool", bufs=1) as qt_pool, \
             tc.tile_pool(name="v_pool", bufs=1) as v_pool, \
             tc.tile_pool(name="at_pool", bufs=1) as at_pool, \
             tc.tile_pool(name="ws_pool", bufs=2) as ws_pool, \
             tc.tile_pool(name="xn_pool", bufs=6) as xn_pool, \
             tc.tile_pool(name="xt_pool", bufs=2) as xt_pool, \
             tc.tile_pool(name="p_pool", bufs=2) as p_pool, \
             tc.tile_pool(name="nrm_pool", bufs=2) as nrm_pool, \
             tc.tile_pool(name="wp_pool", bufs=1) as wp_pool, \
             tc.tile_pool(name="outp", bufs=2) as outp, \
             tc.tile_pool(name="ps1", bufs=4, space="PSUM") as ps1, \
             tc.tile_pool(name="ps2", bufs=2, space="PSUM") as ps2:

            # ---- constants ----
            ident = consts.tile([128, 128], f32)
            make_identity(nc, ident)
            # bqkv as [128, 18]: col j holds bqkv[128j .. 128j+127]
            bq_sb = consts.tile([128, 18], f32)
            nc.sync.dma_start(out=bq_sb, in_=bqkv.rearrange("(j p) -> p j", p=128))
            # bproj broadcast to all partitions
            bp_bc = consts.tile([128, D], f32)
            bp_in = bass.AP(tensor=bproj.tensor, offset=bproj.offset,
                            ap=[[0, 128]] + list(bproj.ap))
            nc.gpsimd.dma_start(out=bp_bc, in_=bp_in)

            # ---- persistent operand tiles ----
            wv = []
            for c in range(KC):
                wvt = wv_pool.tile([128, D], f32r, name=f"wv{c}", tag=f"wv{c}")
                nc.sync.dma_start(
                    out=wvt, in_=wqkv[c * 128:(c + 1) * 128, 2 * D:3 * D].bitcast(f32r))
                wv.append(wvt)
            wp = []
            for c in range(KC):
                wpt = wp_pool.tile([128, D], f32r, name=f"wp{c}", tag=f"wp{c}")
                nc.sync.dma_start(
                    out=wpt, in_=wproj[c * 128:(c + 1) * 128, :].bitcast(f32r))
                wp.append(wpt)
            kt = [kt_pool.tile([128, S], f32r, name=f"kt{j}", tag=f"kt{j}")
                  for j in range(NPAIR)]
            qt = [qt_pool.tile([128, SQ], f32r, name=f"qt{j}", tag=f"qt{j}")
                  for j in range(NPAIR)]
            vt = [v_pool.tile([128, NPAIR * 130], bf16, name=f"vt{k}", tag=f"vt{k}")
                  for k in range(NKC)]
            at = [at_pool.tile([128, SQ], f32r, name=f"at{j}", tag=f"at{j}")
                  for j in range(NPAIR)]

            # ones columns of the V tiles (col 64 and 129 of each pair block)
            for k in range(NKC):
                nc.vector.memset(
                    vt[k].rearrange("p (j t h) -> p j t h", j=NPAIR, t=2)[:, :, :, 64:65],
                    1.0)

            # ---- phase A: x^T, then Q/K/V projections, seq chunk by seq chunk ----
            for s in range(NSC):
                # load x rows and transpose into d-major tiles
                xns = []
                for j in range(4):
                    xn = xn_pool.tile([128, D], f32, name=f"xn{s}_{j}", tag="xn")
                    nc.sync.dma_start(
                        out=xn, in_=xb[s * 512 + j * 128: s * 512 + (j + 1) * 128, :])
                    xns.append(xn)
                xts = []
                for c in range(KC):
                    tp = ps1.tile([128, 512], f32, name=f"tp{s}_{c}", tag="ps1")
                    for j in range(4):
                        nc.tensor.transpose(
                            tp[:, j * 128:(j + 1) * 128],
                            xns[j][:, c * 128:(c + 1) * 128], ident[:])
                    xtc = xt_pool.tile([128, 512], f32r, name=f"xt{s}_{c}", tag=f"xt{c}")
                    nc.vector.tensor_copy(xtc, tp)
                    xts.append(xtc)

                # Q^T (own rows are the first seq chunk thanks to the host-side roll)
                if s == 0:
                    for j in range(NPAIR):
                        wq = ws_pool.tile([128, KC, 128], f32r, name=f"wq{j}", tag="ws")
                        nc.sync.dma_start(
                            out=wq,
                            in_=wqkv[:, j * 128:(j + 1) * 128]
                            .rearrange("(c p) m -> p c m", p=128).bitcast(f32r))
                        qp = ps1.tile([128, 512], f32, name=f"qp{j}", tag="ps1")
                        for c in range(KC):
                            nc.tensor.matmul(qp, wq[:, c, :], xts[c][:],
                                             start=(c == 0), stop=(c == KC - 1))
                        nc.vector.tensor_scalar_add(qt[j], qp, bq_sb[:, j:j + 1])

                # K^T columns for this seq chunk
                for j in range(NPAIR):
                    wk = ws_pool.tile([128, KC, 128], f32r, name=f"wk{j}_{s}", tag="ws")
                    nc.sync.dma_start(
                        out=wk,
                        in_=wqkv[:, D + j * 128:D + (j + 1) * 128]
                        .rearrange("(c p) m -> p c m", p=128).bitcast(f32r))
                    kp = ps1.tile([128, 512], f32, name=f"kp{j}_{s}", tag="ps1")
                    for c in range(KC):
                        nc.tensor.matmul(kp, wk[:, c, :], xts[c][:],
                                         start=(c == 0), stop=(c == KC - 1))
                    nc.vector.tensor_scalar_add(
                        kt[j][:, s * 512:(s + 1) * 512], kp, bq_sb[:, 6 + j:7 + j])

                # V rows for this seq chunk (all heads), into paired bf16 layout
                for m in range(4):
                    k = s * 4 + m
                    vp = ps2.tile([128, 1024], f32, name=f"vp{k}", tag="ps2")
                    for c in range(KC):
                        nc.tensor.matmul(vp[:, 0:512],
                                         xts[c][:, m * 128:(m + 1) * 128],
                                         wv[c][:, 0:512],
                                         start=(c == 0), stop=(c == KC - 1))
                    for c in range(KC):
                        nc.tensor.matmul(vp[:, 512:768],
                                         xts[c][:, m * 128:(m + 1) * 128],
                                         wv[c][:, 512:768],
                                         start=(c == 0), stop=(c == KC - 1))
                    nc.vector.tensor_copy(
                        vt[k].rearrange("p (j t h) -> p j t h",
                                        j=NPAIR, t=2)[:, :, :, 0:64],
                        vp[:, 0:768].rearrange("p (j t h) -> p j t h", j=NPAIR, t=2))

            # ---- phase B: attention, head-pair by head-pair ----
            # The two heads of a pair run as row-tiled concurrent matmuls
            # (lhsT partition bases 0 and 64 -> disjoint PE row strips,
            # outputs in different PSUM banks of one 2-bank tile), and one
            # ACTIVATE exps both heads' scores for the key chunk at once.
            for j in range(NPAIR):
                av_e = ps1.tile([128, 512], f32, name=f"ave{j}", tag="ps1")
                av_o = ps1.tile([128, 512], f32, name=f"avo{j}", tag="ps1")
                for k in range(NKC):
                    sc = ps2.tile([128, 1024], f32, name=f"sc{j}_{k}", tag="ps2")
                    nc.tensor.matmul(sc[:, 0:512],
                                     kt[j][0:64, k * 128:(k + 1) * 128],
                                     qt[j][0:64, :], start=True, stop=True)
                    nc.tensor.matmul(sc[:, 512:1024],
                                     kt[j][64:128, k * 128:(k + 1) * 128],
                                     qt[j][64:128, :], start=True, stop=True)
                    pt = p_pool.tile([128, 1024], bf16, name=f"p{j}_{k}", tag="p")
                    nc.scalar.activation(pt, sc, EXP, scale=SCALE)
                    nc.tensor.matmul(av_e[0:65, :],
                                     vt[k][:, j * 130: j * 130 + 65],
                                     pt[:, 0:512],
                                     start=(k == 0), stop=(k == NKC - 1))
                    nc.tensor.matmul(av_o[0:65, :],
                                     vt[k][:, j * 130 + 65: j * 130 + 130],
                                     pt[:, 512:1024],
                                     start=(k == 0), stop=(k == NKC - 1))
                # normalize into attnout^T (and add v-bias; exact since
                # (V+1 b_v)^T P / sums = V^T P / sums + b_v)
                # Both heads' sums share one reciprocal op: rows 0 and 64
                # (the only partition bases engines accept); unused rows are
                # memset to 1.0 so the reciprocal stays finite.
                sums = nrm_pool.tile([65, 512], f32, name=f"sums{j}", tag="sums", bufs=1)
                nc.gpsimd.memset(sums[:], 1.0)
                nc.vector.tensor_copy(sums[0:1, :], av_e[64:65, :])
                nc.vector.tensor_copy(sums[64:65, :], av_o[64:65, :])
                rs = nrm_pool.tile([65, 512], f32, name=f"rs{j}", tag="rs", bufs=1)
                nc.vector.reciprocal(rs, sums)
                # hw partition_broadcast reads the tile's partition 0, so the
                # odd head's row must first be copied down to a base-0 tile
                rs_o = nrm_pool.tile([1, 512], f32, name=f"rso{j}", tag="rso", bufs=1)
                nc.vector.tensor_copy(rs_o, rs[64:65, :])
                for half, av in ((0, av_e), (1, av_o)):
                    po = half * 64
                    bc = nrm_pool.tile([64, 512], f32, name=f"bc{j}_{half}", tag="bc")
                    nc.gpsimd.partition_broadcast(bc, rs[0:1, :] if half == 0
                                                  else rs_o[:])
                    dst = at[j][po:po + 64, :]
                    nc.vector.tensor_mul(dst, av[0:64, :], bc[:])
                    nc.vector.tensor_scalar_add(dst, dst,
                                                bq_sb[po:po + 64, 12 + j:13 + j])

            # ---- phase C: output projection ----
            for m in range(4):
                pp = ps2.tile([128, 1024], f32, name=f"pp{m}", tag="ps2")
                for c in range(KC):
                    nc.tensor.matmul(pp[:, 0:512],
                                     at[c][:, m * 128:(m + 1) * 128],
                                     wp[c][:, 0:512],
                                     start=(c == 0), stop=(c == KC - 1))
                for c in range(KC):
                    nc.tensor.matmul(pp[:, 512:768],
                                     at[c][:, m * 128:(m + 1) * 128],
                                     wp[c][:, 512:768],
                                     start=(c == 0), stop=(c == KC - 1))
                ot = outp.tile([128, D], f32, name=f"ot{m}", tag="ot")
                nc.vector.tensor_tensor(ot, pp[:, 0:768], bp_bc[:], ADD)
                nc.sync.dma_start(out=out[m * 128:(m + 1) * 128, :], in_=ot)

    nc.compile()
    return nc


def get_nc():
    if "nc" not in _CACHE:
        _CACHE["nc"] = _build()
    return _CACHE["nc"]


def make_in_maps(x, W_qkv, b_qkv, W_proj, b_proj):
    x = np.ascontiguousarray(np.asarray(x, dtype=np.float32))
    W_qkv = np.ascontiguousarray(np.asarray(W_qkv, dtype=np.float32))
    b_qkv = np.ascontiguousarray(np.asarray(b_qkv, dtype=np.float32))
    W_proj = np.ascontiguousarray(np.asarray(W_proj, dtype=np.float32))
    b_proj = np.ascontiguousarray(np.asarray(b_proj, dtype=np.float32))
    in_maps = []
    for c in range(NCORES):
        b, q0 = c // 4, (c % 4) * SQ
        xbat = np.roll(x[b], -q0, axis=0)  # own query rows first; key order is free
        in_maps.append({"xb": np.ascontiguousarray(xbat), "wqkv": W_qkv,
                        "bqkv": b_qkv, "wproj": W_proj, "bproj": b_proj})
    return in_maps


def run(in_maps, **kw):
    return bass_utils.run_bass_kernel_spmd(get_nc(), in_maps,
                                           core_ids=list(range(NCORES)), **kw)


def kernel(x, W_qkv, b_qkv, W_proj, b_proj):
    in_maps = make_in_maps(x, W_qkv, b_qkv, W_proj, b_proj)
    res = run(in_maps)
    out = np.empty((B, N, D), dtype=np.float32)
    for c in range(NCORES):
        b, q0 = c // 4, (c % 4) * SQ
        out[b, q0:q0 + SQ] = res.results[c]["out"]
    return out


# revision 8
# speedup vs baseline: 1.1761x; 1.1448x over previous
"""Multi-head attention (B=2, N=2048, D=768, H=12) on 8 Trainium2 NeuronCores.

Sharding: data-parallel over rows of (B*N) with redundant K/V projection.
Each core c handles batch b=c//4 and query rows q0=(c%4)*512 .. q0+512.
It receives the full batch's x (rolled so its own query rows come first --
softmax over keys is permutation invariant, so key order doesn't matter),
computes K/V for the whole batch (4x redundant) plus Q for its own rows,
runs attention + output projection for its rows, and returns [512, 768].
No cross-core communication is needed.

Device-side layout strategy:
  - x is transposed on the PE (via identity matmuls) into x^T [768, seq]
    d-major tiles, which serve as rhs (K^T/Q^T projections) and lhsT (V).
  - K^T, Q^T are produced directly in [d, seq] layout (fp32r) so the
    scores matmul needs no further transposes; scores come out as
    scores^T [keys, q] in PSUM, exp runs on ACT (scale folded in,
    no max-subtraction needed at these magnitudes), P stored bf16.
  - V tiles are stored bf16 as [V_h0 | 1 | V_h1 | 1] per head-pair so a
    single M=65 matmul per key-chunk yields attnout^T rows 0..63 and the
    softmax denominator (row 64) for free.
  - attnout^T is normalized per head (reciprocal + gpsimd partition
    broadcast) straight into the lhsT layout the output projection needs.
All matmuls run in bf16 (inputs pre-rounded host-side), accumulating in
fp32 PSUM. The softmax scale (1/8) keeps the bf16 score error ~3e-3 in the
exponent, so the end-to-end relative error stays in the few-1e-3 range.
"""

import sys

sys.path.insert(0, "/opt/trn_rl_repo")

import numpy as np

import concourse.bass as bass
import concourse.mybir as mybir
import concourse.tile as tile
from concourse import bacc
from concourse import bass_utils
from concourse.masks import make_identity

B, N, D = 2, 2048, 768
H, DH = 12, 64
NCORES = 8
S = 2048          # keys per batch
SQ = 512          # query rows per core
NSC = S // 512    # seq chunks (of 512) for the projections
NKC = S // 128    # key chunks (of 128) for attention
NPAIR = H // 2    # head pairs
KC = D // 128     # contraction chunks
SCALE = DH ** -0.5

f32 = mybir.dt.float32
f32r = mybir.dt.float32r
bf16 = mybir.dt.bfloat16
ADD = mybir.AluOpType.add
EXP = mybir.ActivationFunctionType.Exp

_CACHE = {}


def _build():
    nc = bacc.Bacc("TRN2", target_bir_lowering=False, debug=False,
                   enable_asserts=False, num_devices=NCORES)
    xb = nc.dram_tensor("xb", [S, D], bf16, kind="ExternalInput").ap()
    wqkv = nc.dram_tensor("wqkv", [D, 3 * D], bf16, kind="ExternalInput").ap()
    bqkv = nc.dram_tensor("bqkv", [3 * D], f32, kind="ExternalInput").ap()
    wproj = nc.dram_tensor("wproj", [D, D], bf16, kind="ExternalInput").ap()
    bproj = nc.dram_tensor("bproj", [D], f32, kind="ExternalInput").ap()
    out = nc.dram_tensor("out", [SQ, D], f32, kind="ExternalOutput").ap()

    with tile.TileContext(nc) as tc:
        with tc.tile_pool(name="consts", bufs=1) as consts, \
             tc.tile_pool(name="wv_pool", bufs=1) as wv_pool, \
             tc.tile_pool(name="kt_pool", bufs=1) as kt_pool, \
             tc.tile_pool(name="qt_pool", bufs=1) as qt_pool, \
             tc.tile_pool(name="v_pool", bufs=1) as v_pool, \
             tc.tile_pool(name="at_pool", bufs=1) as at_pool, \
             tc.tile_pool(name="wq_pool", bufs=1) as wq_pool, \
             tc.tile_pool(name="wk_pool", bufs=1) as wk_pool, \
             tc.tile_pool(name="xn_pool", bufs=8) as xn_pool, \
             tc.tile_pool(name="xt_pool", bufs=2) as xt_pool, \
             tc.tile_pool(name="p_pool", bufs=3) as p_pool, \
             tc.tile_pool(name="nrm_pool", bufs=2) as nrm_pool, \
             tc.tile_pool(name="wp_pool", bufs=1) as wp_pool, \
             tc.tile_pool(name="outp", bufs=2) as outp, \
             tc.tile_pool(name="ps1", bufs=4, space="PSUM") as ps1, \
             tc.tile_pool(name="ps2", bufs=2, space="PSUM") as ps2:

            # ---- constants ----
            ident = consts.tile([128, 128], bf16)
            make_identity(nc, ident)
            # bqkv as [128, 18]: col j holds bqkv[128j .. 128j+127]
            bq_sb = consts.tile([128, 18], f32)
            nc.sync.dma_start(out=bq_sb, in_=bqkv.rearrange("(j p) -> p j", p=128))
            # bproj broadcast to all partitions
            bp_bc = consts.tile([128, D], f32)
            bp_in = bass.AP(tensor=bproj.tensor, offset=bproj.offset,
                            ap=[[0, 128]] + list(bproj.ap))
            nc.gpsimd.dma_start(out=bp_bc, in_=bp_in)

            # ---- persistent operand tiles ----
            wq, wk, wv, wp = [], [], [], []
            for c in range(KC):
                rows = slice(c * 128, (c + 1) * 128)
                wqt = wq_pool.tile([128, D], bf16, name=f"wq{c}", tag=f"wq{c}")
                nc.sync.dma_start(out=wqt, in_=wqkv[rows, 0:D])
                wq.append(wqt)
                wkt = wk_pool.tile([128, D], bf16, name=f"wk{c}", tag=f"wk{c}")
                nc.sync.dma_start(out=wkt, in_=wqkv[rows, D:2 * D])
                wk.append(wkt)
                wvt = wv_pool.tile([128, D], bf16, name=f"wv{c}", tag=f"wv{c}")
                nc.sync.dma_start(out=wvt, in_=wqkv[rows, 2 * D:3 * D])
                wv.append(wvt)
                wpt = wp_pool.tile([128, D], bf16, name=f"wp{c}", tag=f"wp{c}")
                nc.sync.dma_start(out=wpt, in_=wproj[rows, :])
                wp.append(wpt)
            kt = [kt_pool.tile([128, S], bf16, name=f"kt{j}", tag=f"kt{j}")
                  for j in range(NPAIR)]
            qt = [qt_pool.tile([128, SQ], bf16, name=f"qt{j}", tag=f"qt{j}")
                  for j in range(NPAIR)]
            vt = [v_pool.tile([128, NPAIR * 130], bf16, name=f"vt{k}", tag=f"vt{k}")
                  for k in range(NKC)]
            at = [at_pool.tile([128, SQ], bf16, name=f"at{j}", tag=f"at{j}")
                  for j in range(NPAIR)]

            # ones columns of the V tiles (col 64 and 129 of each pair block)
            for k in range(NKC):
                nc.vector.memset(
                    vt[k].rearrange("p (j t h) -> p j t h", j=NPAIR, t=2)[:, :, :, 64:65],
                    1.0)

            # ---- phase A: x^T, then Q/K/V projections, seq chunk by seq chunk ----
            for s in range(NSC):
                # load x rows and transpose into d-major tiles
                xns = []
                for j in range(4):
                    xn = xn_pool.tile([128, D], bf16, name=f"xn{s}_{j}", tag="xn")
                    nc.sync.dma_start(
                        out=xn, in_=xb[s * 512 + j * 128: s * 512 + (j + 1) * 128, :])
                    xns.append(xn)
                xts = []
                for c in range(KC):
                    tp = ps1.tile([128, 512], bf16, name=f"tp{s}_{c}", tag="ps1")
                    for j in range(4):
                        nc.tensor.transpose(
                            tp[:, j * 128:(j + 1) * 128],
                            xns[j][:, c * 128:(c + 1) * 128], ident[:])
                    xtc = xt_pool.tile([128, 512], bf16, name=f"xt{s}_{c}", tag=f"xt{c}")
                    nc.vector.tensor_copy(xtc, tp)
                    xts.append(xtc)

                # Q^T (own rows are the first seq chunk thanks to the host-side roll)
                if s == 0:
                    for j in range(NPAIR):
                        qp = ps1.tile([128, 512], f32, name=f"qp{j}", tag="ps1")
                        for c in range(KC):
                            nc.tensor.matmul(qp, wq[c][:, j * 128:(j + 1) * 128],
                                             xts[c][:],
                                             start=(c == 0), stop=(c == KC - 1))
                        nc.vector.tensor_scalar_add(qt[j], qp, bq_sb[:, j:j + 1])

                # K^T columns for this seq chunk
                for j in range(NPAIR):
                    kp = ps1.tile([128, 512], f32, name=f"kp{j}_{s}", tag="ps1")
                    for c in range(KC):
                        nc.tensor.matmul(kp, wk[c][:, j * 128:(j + 1) * 128],
                                         xts[c][:],
                                         start=(c == 0), stop=(c == KC - 1))
                    nc.vector.tensor_scalar_add(
                        kt[j][:, s * 512:(s + 1) * 512], kp, bq_sb[:, 6 + j:7 + j])

                # V rows for this seq chunk (all heads), into paired bf16 layout
                for m in range(4):
                    k = s * 4 + m
                    vp = ps2.tile([128, 1024], f32, name=f"vp{k}", tag="ps2")
                    for c in range(KC):
                        nc.tensor.matmul(vp[:, 0:512],
                                         xts[c][:, m * 128:(m + 1) * 128],
                                         wv[c][:, 0:512],
                                         start=(c == 0), stop=(c == KC - 1))
                    for c in range(KC):
                        nc.tensor.matmul(vp[:, 512:768],
                                         xts[c][:, m * 128:(m + 1) * 128],
                                         wv[c][:, 512:768],
                                         start=(c == 0), stop=(c == KC - 1))
                    nc.vector.tensor_copy(
                        vt[k].rearrange("p (j t h) -> p j t h",
                                        j=NPAIR, t=2)[:, :, :, 0:64],
                        vp[:, 0:768].rearrange("p (j t h) -> p j t h", j=NPAIR, t=2))

            # ---- phase B: attention, head-pair by head-pair ----
            # The two heads of a pair run as row-tiled concurrent matmuls
            # (lhsT partition bases 0 and 64 -> disjoint PE row strips,
            # outputs in different PSUM banks of one 2-bank tile), and one
            # ACTIVATE exps both heads' scores for the key chunk at once.
            for j in range(NPAIR):
                av_e = ps1.tile([128, 512], f32, name=f"ave{j}", tag="ps1")
                av_o = ps1.tile([128, 512], f32, name=f"avo{j}", tag="ps1")
                for k in range(NKC):
                    sc = ps2.tile([128, 1024], f32, name=f"sc{j}_{k}", tag="ps2")
                    nc.tensor.matmul(sc[:, 0:512],
                                     kt[j][0:64, k * 128:(k + 1) * 128],
                                     qt[j][0:64, :], start=True, stop=True)
                    nc.tensor.matmul(sc[:, 512:1024],
                                     kt[j][64:128, k * 128:(k + 1) * 128],
                                     qt[j][64:128, :], start=True, stop=True)
                    pt = p_pool.tile([128, 1024], bf16, name=f"p{j}_{k}", tag="p")
                    nc.scalar.activation(pt, sc, EXP, scale=SCALE)
                    nc.tensor.matmul(av_e[0:65, :],
                                     vt[k][:, j * 130: j * 130 + 65],
                                     pt[:, 0:512],
                                     start=(k == 0), stop=(k == NKC - 1))
                    nc.tensor.matmul(av_o[0:65, :],
                                     vt[k][:, j * 130 + 65: j * 130 + 130],
                                     pt[:, 512:1024],
                                     start=(k == 0), stop=(k == NKC - 1))
                # normalize into attnout^T (and add v-bias; exact since
                # (V+1 b_v)^T P / sums = V^T P / sums + b_v)
                # Both heads' sums share one reciprocal op: rows 0 and 64
                # (the only partition bases engines accept); unused rows are
                # memset to 1.0 so the reciprocal stays finite.
                sums = nrm_pool.tile([65, 512], f32, name=f"sums{j}", tag="sums", bufs=1)
                nc.gpsimd.memset(sums[:], 1.0)
                nc.vector.tensor_copy(sums[0:1, :], av_e[64:65, :])
                nc.vector.tensor_copy(sums[64:65, :], av_o[64:65, :])
                rs = nrm_pool.tile([65, 512], f32, name=f"rs{j}", tag="rs", bufs=1)
                nc.vector.reciprocal(rs, sums)
                # hw partition_broadcast reads the tile's partition 0, so the
                # odd head's row must first be copied down to a base-0 tile
                rs_o = nrm_pool.tile([1, 512], f32, name=f"rso{j}", tag="rso", bufs=1)
                nc.vector.tensor_copy(rs_o, rs[64:65, :])
                for half, av in ((0, av_e), (1, av_o)):
                    po = half * 64
                    bc = nrm_pool.tile([64, 512], f32, name=f"bc{j}_{half}", tag="bc")
                    nc.gpsimd.partition_broadcast(bc, rs[0:1, :] if half == 0
                                                  else rs_o[:])
                    dst = at[j][po:po + 64, :]
                    nc.vector.tensor_mul(dst, av[0:64, :], bc[:])
                    nc.vector.tensor_scalar_add(dst, dst,
                                                bq_sb[po:po + 64, 12 + j:13 + j])

            # ---- phase C: output projection ----
            for m in range(4):
                pp = ps2.tile([128, 1024], f32, name=f"pp{m}", tag="ps2")
                for c in range(KC):
                    nc.tensor.matmul(pp[:, 0:512],
                                     at[c][:, m * 128:(m + 1) * 128],
                                     wp[c][:, 0:512],
                                     start=(c == 0), stop=(c == KC - 1))
                for c in range(KC):
                    nc.tensor.matmul(pp[:, 512:768],
                                     at[c][:, m * 128:(m + 1) * 128],
                                     wp[c][:, 512:768],
                                     start=(c == 0), stop=(c == KC - 1))
                ot = outp.tile([128, D], f32, name=f"ot{m}", tag="ot")
                nc.vector.tensor_tensor(ot, pp[:, 0:768], bp_bc[:], ADD)
                nc.sync.dma_start(out=out[m * 128:(m + 1) * 128, :], in_=ot)

    nc.compile()
    return nc


def get_nc():
    if "nc" not in _CACHE:
        _CACHE["nc"] = _build()
    return _CACHE["nc"]


def make_in_maps(x, W_qkv, b_qkv, W_proj, b_proj):
    import ml_dtypes
    bf = ml_dtypes.bfloat16
    x = np.ascontiguousarray(np.asarray(x, dtype=np.float32).astype(bf))
    W_qkv = np.ascontiguousarray(np.asarray(W_qkv, dtype=np.float32).astype(bf))
    b_qkv = np.ascontiguousarray(np.asarray(b_qkv, dtype=np.float32))
    W_proj = np.ascontiguousarray(np.asarray(W_proj, dtype=np.float32).astype(bf))
    b_proj = np.ascontiguousarray(np.asarray(b_proj, dtype=np.float32))
    in_maps = []
    for c in range(NCORES):
        b, q0 = c // 4, (c % 4) * SQ
        xbat = np.roll(x[b], -q0, axis=0)  # own query rows first; key order is free
        in_maps.append({"xb": np.ascontiguousarray(xbat), "wqkv": W_qkv,
                        "bqkv": b_qkv, "wproj": W_proj, "bproj": b_proj})
    return in_maps


def run(in_maps, **kw):
    return bass_utils.run_bass_kernel_spmd(get_nc(), in_maps,
                                           core_ids=list(range(NCORES)), **kw)


def kernel(x, W_qkv, b_qkv, W_proj, b_proj):
    in_maps = make_in_maps(x, W_qkv, b_qkv, W_proj, b_proj)
    res = run(in_maps)
    out = np.empty((B, N, D), dtype=np.float32)
    for c in range(NCORES):
        b, q0 = c // 4, (c % 4) * SQ
        out[b, q0:q0 + SQ] = res.results[c]["out"]
    return out
